# revision 4
# baseline (speedup 1.0000x reference)
"""Trainium2 Bass kernel for nn_DotProductAttention (B=8, LQ=LK=4096, F=64).

Reference computation:
    q = query @ wq.T + bq ; k = key @ wk.T + bk ; v = value @ wv.T + bv
    scores = einsum('bkf,bqf->bkq', k, q)
    attn = softmax(scores, axis=-1)           # over q positions
    out = einsum('bkq,bqf->bkf', attn, v)

Strategy: batch b -> core b (8 cores, no cross-core communication).

Algebraic folding (host side):
    scores[k,q] = (wk x_k + bk) . (wq x_q + bq)
                = x_q^T (wq^T wk) x_k + x_q^T (wq^T bk) + [per-k term]
    The per-k term is constant along the softmax axis (q) and cancels, so with
    M = wq^T wk and c = wq^T bk the effective transposed scores are
        S^T[q,k] = query[q,:] @ ktil[:,k],  ktil = M @ key^T + c.
    Softmax rows sum to 1, so the v-projection commutes with attention:
        out = (attn @ value) @ wv.T + bv
    exp() is applied without max-subtraction: |S| < ~70 so exp fits fp32/bf16.
    U^T = [value | 1]^T @ exp(S^T) accumulates in PSUM; its last row is the
    softmax denominator l. The output projection uses W = [wv.T; bv | e64] so
    column 64 of the product is l, and a per-partition reciprocal multiply
    normalizes.

Per-core engine budget: ACT exp over 16.7M elements (~135us) bounds; PE
matmuls (scores row-tiled over two 64-row groups, P@V with M=65) fit under.
"""

import numpy as np

import concourse.bass as bass
import concourse.mybir as mybir
import concourse.tile as tile
from concourse import bacc
from concourse.bass_utils import run_bass_kernel_spmd
from concourse.masks import make_identity

F32 = mybir.dt.float32
F16 = mybir.dt.float16
BF16 = mybir.dt.bfloat16

L = 4096          # sequence length (both q and k)
F = 64            # feature dim
NBLK = L // 128   # 32 position blocks
NPAIR = NBLK // 2  # 16 pair tiles
NCH = 8           # 512-wide k chunks
CHW = 512


def kblock_of(c, i):
    """k-block index written by chunk c, sub-block i (see pair-transpose layout)."""
    cc = c % 4
    return 2 * (4 * cc + i) + (1 if c >= 4 else 0)


def build_nc():
    nc = bacc.Bacc(None, target_bir_lowering=False)

    query = nc.dram_tensor("query", [L, F], F32, kind="ExternalInput")
    key = nc.dram_tensor("key", [L, F], F32, kind="ExternalInput")
    value = nc.dram_tensor("value", [L, F], F32, kind="ExternalInput")
    # mT2: rows 0-63 and 64-127 both hold (wq.T @ wk).T
    mT2 = nc.dram_tensor("mT2", [128, F], F32, kind="ExternalInput")
    # cvec2: (wq.T @ bk) duplicated into both partition halves
    cvec2 = nc.dram_tensor("cvec2", [128, 1], F32, kind="ExternalInput")
    # wvb: rows 0-63 wv.T, row 64 = [bv | 1], rows 65-127 zero
    wvb = nc.dram_tensor("wvb", [128, F + 1], F32, kind="ExternalInput")
    out = nc.dram_tensor("out", [L, F], F32, kind="ExternalOutput")

    Exp = mybir.ActivationFunctionType.Exp
    Mult = mybir.AluOpType.mult

    with tile.TileContext(nc) as tc:
        with (
            tc.tile_pool(name="consts", bufs=1) as consts,
            tc.tile_pool(name="persist", bufs=1) as persist,
            tc.tile_pool(name="stage", bufs=4) as stage,
            tc.tile_pool(name="pt", bufs=3) as ptpool,
            tc.tile_pool(name="utb", bufs=2) as utbpool,
            tc.tile_pool(name="osb", bufs=4) as osbpool,
            tc.tile_pool(name="rc", bufs=4) as rcpool,
            tc.tile_pool(name="ps_misc", bufs=2, space="PSUM") as ps_misc,
            tc.tile_pool(name="ps_st", bufs=2, space="PSUM") as ps_st,
            tc.tile_pool(name="ps_ut", bufs=2, space="PSUM") as ps_ut,
        ):
            # ---- constants ----
            ident = consts.tile([128, 128], F32)
            make_identity(nc, ident)
            mT2_f32 = consts.tile([128, F], F32)
            nc.sync.dma_start(mT2_f32[:], mT2[:])
            mT2_sb = consts.tile([128, F], F16)
            nc.vector.tensor_copy(mT2_sb[:], mT2_f32[:])
            cvec_sb = consts.tile([128, 1], F32)
            nc.sync.dma_start(cvec_sb[:], cvec2[:])
            wvb_f32 = consts.tile([128, F + 1], F32)
            nc.sync.dma_start(wvb_f32[:], wvb[:])
            wvb_sb = consts.tile([128, F + 1], BF16)
            nc.vector.tensor_copy(wvb_sb[:], wvb_f32[:])

            # ---- persistent sbuf panels ----
            xkT = persist.tile([128, NPAIR * 128], F16)   # key^T pair layout
            xqT = persist.tile([128, NPAIR * 128], F16)   # query^T pair layout
            ktil = persist.tile([128, L], F16)            # M @ key^T + c (dup halves)
            vaug = persist.tile([128, NBLK * (F + 1)], BF16)  # [value | 1] per block

            # ---- key transposes ----
            for t in range(NPAIR):
                stg = stage.tile([128, 128], F32, tag="qkstage")
                nc.sync.dma_start(stg[:, 0:F], key[256 * t: 256 * t + 128, :])
                nc.sync.dma_start(stg[:, F:128], key[256 * t + 128: 256 * t + 256, :])
                tp = ps_misc.tile([128, 128], F32, tag="misc")
                nc.tensor.transpose(tp[:], stg[:], ident[:])
                nc.vector.tensor_copy(xkT[:, 128 * t: 128 * (t + 1)], tp[:])

            # ---- ktil projection (col-tiled duplication into both halves) ----
            for c in range(NCH):
                rh = 0 if c < 4 else 64
                cc = c % 4
                rhs = xkT[rh:rh + 64, CHW * cc: CHW * (cc + 1)]
                kt_ps = ps_misc.tile([128, CHW], F32, tag="misc")
                nc.tensor.matmul(kt_ps[0:64, :], mT2_sb[rh:rh + 64, :], rhs,
                                 start=True, stop=True, tile_position=(rh, 0))
                nc.tensor.matmul(kt_ps[64:128, :], mT2_sb[rh:rh + 64, :], rhs,
                                 start=True, stop=True, tile_position=(rh, 64))
                nc.vector.tensor_scalar_add(
                    ktil[:, CHW * c: CHW * (c + 1)], kt_ps[:], cvec_sb[:])

            # ---- query transposes ----
            for t in range(NPAIR):
                stg = stage.tile([128, 128], F32, tag="qkstage")
                nc.sync.dma_start(stg[:, 0:F], query[256 * t: 256 * t + 128, :])
                nc.sync.dma_start(stg[:, F:128], query[256 * t + 128: 256 * t + 256, :])
                tp = ps_misc.tile([128, 128], F32, tag="misc")
                nc.tensor.transpose(tp[:], stg[:], ident[:])
                nc.vector.tensor_copy(xqT[:, 128 * t: 128 * (t + 1)], tp[:])

            # ---- value staging ----
            for b in range(NBLK):
                vstg = stage.tile([128, F], F32, tag="vstage")
                nc.sync.dma_start(vstg[:], value[128 * b: 128 * (b + 1), :])
                o = (F + 1) * b
                nc.vector.tensor_copy(vaug[:, o: o + F], vstg[:])
                nc.vector.memset(vaug[:, o + F: o + F + 1], 1.0)

            # ---- main loop ----
            for c in range(NCH):
                kcols = slice(CHW * c, CHW * (c + 1))
                ut = ps_ut.tile([F + 1, CHW], F32)
                for jp in range(NPAIR):
                    qcols = slice(128 * jp, 128 * (jp + 1))
                    st = ps_st.tile([128, 1024], F32)
                    nc.tensor.matmul(st[:, 0:512], xqT[0:64, qcols],
                                     ktil[0:64, kcols],
                                     start=True, stop=True, tile_position=(0, 0))
                    nc.tensor.matmul(st[:, 512:1024], xqT[64:128, qcols],
                                     ktil[64:128, kcols],
                                     start=True, stop=True, tile_position=(64, 0))
                    pt = ptpool.tile([128, 1024], BF16)
                    nc.scalar.activation(pt[:], st[:], Exp)
                    ja = (F + 1) * (2 * jp)
                    jb = (F + 1) * (2 * jp + 1)
                    nc.tensor.matmul(ut[:], vaug[:, ja: ja + F + 1], pt[:, 0:512],
                                     start=(jp == 0), stop=False)
                    nc.tensor.matmul(ut[:], vaug[:, jb: jb + F + 1], pt[:, 512:1024],
                                     start=False, stop=(jp == NPAIR - 1))

                # epilogue: bf16 U^T -> out projection -> normalize -> DMA
                utb = utbpool.tile([128, CHW], BF16)
                nc.vector.memset(utb[F:128, :], 0.0)
                nc.vector.tensor_copy(utb[0:F + 1, :], ut[:])
                for i in range(4):
                    ops = ps_misc.tile([128, F + 1], F32, tag="misc")
                    nc.tensor.matmul(ops[:], utb[:, 128 * i: 128 * (i + 1)],
                                     wvb_sb[:], start=True, stop=True)
                    rc = rcpool.tile([128, 1], F32)
                    nc.vector.reciprocal(rc[:], ops[:, F:F + 1])
                    osb = osbpool.tile([128, F], F32)
                    nc.vector.tensor_scalar_mul(osb[:], ops[:, 0:F], rc[:])
                    kb = kblock_of(c, i)
                    nc.sync.dma_start(out[128 * kb: 128 * (kb + 1), :], osb[:])

    nc.compile()
    return nc


def host_consts(wq, bq, wk, bk, wv, bv):
    wq64 = wq.astype(np.float64)
    M = (wq64.T @ wk.astype(np.float64)).astype(np.float32)
    c = (wq64.T @ bk.astype(np.float64)).astype(np.float32)
    mT2 = np.concatenate([M.T, M.T], axis=0).astype(np.float32)        # [128, 64]
    cvec2 = np.concatenate([c, c])[:, None].astype(np.float32)         # [128, 1]
    wvb = np.zeros((128, F + 1), np.float32)
    wvb[0:F, 0:F] = wv.T
    wvb[F, 0:F] = bv
    wvb[F, F] = 1.0
    return mT2, cvec2, wvb


_NC = None


def kernel(**inputs):
    out, _ = run_kernel(inputs)
    return out


def run_kernel(inputs, **spmd_kwargs):
    global _NC
    if _NC is None:
        _NC = build_nc()

    query = np.ascontiguousarray(np.asarray(inputs["query"], np.float32))
    key = np.ascontiguousarray(np.asarray(inputs["key"], np.float32))
    value = np.ascontiguousarray(np.asarray(inputs["value"], np.float32))
    mT2, cvec2, wvb = host_consts(
        np.asarray(inputs["wq"], np.float32), np.asarray(inputs["bq"], np.float32),
        np.asarray(inputs["wk"], np.float32), np.asarray(inputs["bk"], np.float32),
        np.asarray(inputs["wv"], np.float32), np.asarray(inputs["bv"], np.float32))

    B = query.shape[0]
    in_maps = [
        {
            "query": query[b], "key": key[b], "value": value[b],
            "mT2": mT2, "cvec2": cvec2, "wvb": wvb,
        }
        for b in range(B)
    ]
    res = run_bass_kernel_spmd(_NC, in_maps, core_ids=list(range(B)), **spmd_kwargs)
    out = np.stack([res.results[b]["out"] for b in range(B)]).astype(np.float32)
    return out, res


# revision 6
# speedup vs baseline: 1.1021x; 1.1021x over previous
"""Trainium2 Bass kernel for nn_DotProductAttention (B=8, LQ=LK=4096, F=64).

Reference computation:
    q = query @ wq.T + bq ; k = key @ wk.T + bk ; v = value @ wv.T + bv
    scores = einsum('bkf,bqf->bkq', k, q)
    attn = softmax(scores, axis=-1)           # over q positions
    out = einsum('bkq,bqf->bkf', attn, v)

Strategy: batch b -> core b (8 cores, no cross-core communication).

Algebraic folding (host side):
    scores[k,q] = (wk x_k + bk) . (wq x_q + bq)
                = x_q^T (wq^T wk) x_k + x_q^T (wq^T bk) + [per-k term]
    The per-k term is constant along the softmax axis (q) and cancels, so with
    M = wq^T wk and c = wq^T bk the effective transposed scores are
        S^T[q,k] = query[q,:] @ ktil[:,k],  ktil = M @ key^T + c.
    Softmax rows sum to 1, so the v-projection commutes with attention:
        out = (attn @ value) @ wv.T + bv
    exp() is applied without max-subtraction: |S| < ~70 so exp fits fp32/bf16.
    U^T = [value | 1]^T @ exp(S^T) accumulates in PSUM; its last row is the
    softmax denominator l. The output projection uses W = [wv.T; bv | e64] so
    column 64 of the product is l, and a per-partition reciprocal multiply
    normalizes.

Per-core engine budget: ACT exp over 16.7M elements (~135us) bounds; PE
matmuls (scores row-tiled over two 64-row groups, P@V with M=65) fit under.
"""

import numpy as np

import concourse.bass as bass
import concourse.mybir as mybir
import concourse.tile as tile
from concourse import bacc
from concourse.bass_utils import run_bass_kernel_spmd
from concourse.masks import make_identity

F32 = mybir.dt.float32
F16 = mybir.dt.float16
BF16 = mybir.dt.bfloat16

L = 4096          # sequence length (both q and k)
F = 64            # feature dim
NBLK = L // 128   # 32 position blocks
NPAIR = NBLK // 2  # 16 pair tiles
NCH = 8           # 512-wide k chunks
CHW = 512


def kblock_of(c, i):
    """k-block index written by chunk c, sub-block i (see pair-transpose layout)."""
    cc = c % 4
    return 2 * (4 * cc + i) + (1 if c >= 4 else 0)


def build_nc():
    nc = bacc.Bacc(None, target_bir_lowering=False)

    query = nc.dram_tensor("query", [L, F], F32, kind="ExternalInput")
    key = nc.dram_tensor("key", [L, F], F32, kind="ExternalInput")
    value = nc.dram_tensor("value", [L, F], F32, kind="ExternalInput")
    # mT2: rows 0-63 and 64-127 both hold (wq.T @ wk).T
    mT2 = nc.dram_tensor("mT2", [128, F], F32, kind="ExternalInput")
    # cvec2: (wq.T @ bk) duplicated into both partition halves
    cvec2 = nc.dram_tensor("cvec2", [128, 1], F32, kind="ExternalInput")
    # wvb: rows 0-63 wv.T, row 64 = [bv | 1], rows 65-127 zero
    wvb = nc.dram_tensor("wvb", [128, F + 1], F32, kind="ExternalInput")
    out = nc.dram_tensor("out", [L, F], F32, kind="ExternalOutput")

    Exp = mybir.ActivationFunctionType.Exp
    Mult = mybir.AluOpType.mult

    with tile.TileContext(nc) as tc:
        with (
            tc.tile_pool(name="consts", bufs=1) as consts,
            tc.tile_pool(name="persist", bufs=1) as persist,
            tc.tile_pool(name="stage", bufs=4) as stage,
            tc.tile_pool(name="pt", bufs=3) as ptpool,
            tc.tile_pool(name="utb", bufs=2) as utbpool,
            tc.tile_pool(name="osb", bufs=4) as osbpool,
            tc.tile_pool(name="rc", bufs=4) as rcpool,
            tc.tile_pool(name="ps_misc", bufs=2, space="PSUM") as ps_misc,
            tc.tile_pool(name="ps_st", bufs=2, space="PSUM") as ps_st,
            tc.tile_pool(name="ps_ut", bufs=2, space="PSUM") as ps_ut,
        ):
            # ---- constants ----
            ident = consts.tile([128, 128], F32)
            make_identity(nc, ident)
            mT2_f32 = consts.tile([128, F], F32)
            nc.sync.dma_start(mT2_f32[:], mT2[:])
            mT2_sb = consts.tile([128, F], F16)
            nc.vector.tensor_copy(mT2_sb[:], mT2_f32[:])
            cvec_sb = consts.tile([128, 1], F32)
            nc.sync.dma_start(cvec_sb[:], cvec2[:])
            wvb_f32 = consts.tile([128, F + 1], F32)
            nc.sync.dma_start(wvb_f32[:], wvb[:])
            wvb_sb = consts.tile([128, F + 1], BF16)
            nc.vector.tensor_copy(wvb_sb[:], wvb_f32[:])

            # ---- persistent sbuf panels ----
            xkT = persist.tile([128, NPAIR * 128], F16)   # key^T pair layout
            xqT = persist.tile([128, NPAIR * 128], F16)   # query^T pair layout
            ktil = persist.tile([128, L], F16)            # M @ key^T + c (dup halves)
            vaug = persist.tile([128, NBLK * (F + 1)], BF16)  # [value | 1] per block

            # ---- batched input DMAs (one per tensor; pair layout for q/k) ----
            kstage = stage.tile([128, NPAIR, 2, F], F32, tag="kstage")
            nc.sync.dma_start(
                kstage[:], key.rearrange("(t h p) f -> p t h f", p=128, h=2))
            vstage = stage.tile([128, NBLK, F], F32, tag="vstage")
            nc.sync.dma_start(
                vstage[:], value.rearrange("(b p) f -> p b f", p=128))
            qstage = stage.tile([128, NPAIR, 2, F], F32, tag="qstage")
            nc.sync.dma_start(
                qstage[:], query.rearrange("(t h p) f -> p t h f", p=128, h=2))

            # ---- key transposes ----
            for t in range(NPAIR):
                tp = ps_misc.tile([128, 128], F32, tag="misc")
                nc.tensor.transpose(tp[:], kstage[:, t], ident[:])
                nc.vector.tensor_copy(xkT[:, 128 * t: 128 * (t + 1)], tp[:])

            # ---- ktil projection (col-tiled duplication into both halves) ----
            for c in range(NCH):
                rh = 0 if c < 4 else 64
                cc = c % 4
                rhs = xkT[rh:rh + 64, CHW * cc: CHW * (cc + 1)]
                kt_ps = ps_misc.tile([128, CHW], F32, tag="misc")
                nc.tensor.matmul(kt_ps[0:64, :], mT2_sb[rh:rh + 64, :], rhs,
                                 start=True, stop=True, tile_position=(rh, 0))
                nc.tensor.matmul(kt_ps[64:128, :], mT2_sb[rh:rh + 64, :], rhs,
                                 start=True, stop=True, tile_position=(rh, 64))
                nc.vector.tensor_scalar_add(
                    ktil[:, CHW * c: CHW * (c + 1)], kt_ps[:], cvec_sb[:])

            # ---- query transposes ----
            for t in range(NPAIR):
                tp = ps_misc.tile([128, 128], F32, tag="misc")
                nc.tensor.transpose(tp[:], qstage[:, t], ident[:])
                nc.vector.tensor_copy(xqT[:, 128 * t: 128 * (t + 1)], tp[:])

            # ---- value staging ----
            for b in range(NBLK):
                o = (F + 1) * b
                nc.vector.tensor_copy(vaug[:, o: o + F], vstage[:, b])
                nc.vector.memset(vaug[:, o + F: o + F + 1], 1.0)

            # ---- main loop, software-pipelined: scores(i+1) before pav(i) ----
            iters = [(c, jp) for c in range(NCH) for jp in range(NPAIR)]

            def emit_scores(c, jp):
                kcols = slice(CHW * c, CHW * (c + 1))
                qcols = slice(128 * jp, 128 * (jp + 1))
                st = ps_st.tile([128, 1024], F32, name="st", tag="st")
                nc.tensor.matmul(st[:, 0:512], xqT[0:64, qcols],
                                 ktil[0:64, kcols],
                                 start=True, stop=True, tile_position=(0, 0))
                nc.tensor.matmul(st[:, 512:1024], xqT[64:128, qcols],
                                 ktil[64:128, kcols],
                                 start=True, stop=True, tile_position=(64, 0))
                pt = ptpool.tile([128, 1024], BF16, name="pt", tag="pt")
                nc.scalar.activation(pt[:], st[:], Exp)
                return pt

            uts = {}

            def emit_pav(c, jp, pt):
                if jp == 0:
                    uts[c] = ps_ut.tile([F + 1, CHW], F32, name="ut", tag="ut")
                ja = (F + 1) * (2 * jp)
                jb = (F + 1) * (2 * jp + 1)
                nc.tensor.matmul(uts[c][:], vaug[:, ja: ja + F + 1], pt[:, 0:512],
                                 start=(jp == 0), stop=False)
                nc.tensor.matmul(uts[c][:], vaug[:, jb: jb + F + 1],
                                 pt[:, 512:1024],
                                 start=False, stop=(jp == NPAIR - 1))

            def emit_epilogue(c):
                ut = uts.pop(c)
                utb = utbpool.tile([128, CHW], BF16)
                nc.vector.memset(utb[F:128, :], 0.0)
                nc.vector.tensor_copy(utb[0:F + 1, :], ut[:])
                for i in range(4):
                    ops = ps_misc.tile([128, F + 1], F32, tag="misc")
                    nc.tensor.matmul(ops[:], utb[:, 128 * i: 128 * (i + 1)],
                                     wvb_sb[:], start=True, stop=True)
                    rc = rcpool.tile([128, 1], F32)
                    nc.vector.reciprocal(rc[:], ops[:, F:F + 1])
                    osb = osbpool.tile([128, F], F32)
                    nc.vector.tensor_scalar_mul(osb[:], ops[:, 0:F], rc[:])
                    kb = kblock_of(c, i)
                    nc.sync.dma_start(out[128 * kb: 128 * (kb + 1), :], osb[:])

            pts = {0: emit_scores(*iters[0])}
            for idx, (c, jp) in enumerate(iters):
                if idx + 1 < len(iters):
                    pts[idx + 1] = emit_scores(*iters[idx + 1])
                emit_pav(c, jp, pts.pop(idx))
                if jp == NPAIR - 1:
                    emit_epilogue(c)

    nc.compile()
    return nc


def host_consts(wq, bq, wk, bk, wv, bv):
    wq64 = wq.astype(np.float64)
    M = (wq64.T @ wk.astype(np.float64)).astype(np.float32)
    c = (wq64.T @ bk.astype(np.float64)).astype(np.float32)
    mT2 = np.concatenate([M.T, M.T], axis=0).astype(np.float32)        # [128, 64]
    cvec2 = np.concatenate([c, c])[:, None].astype(np.float32)         # [128, 1]
    wvb = np.zeros((128, F + 1), np.float32)
    wvb[0:F, 0:F] = wv.T
    wvb[F, 0:F] = bv
    wvb[F, F] = 1.0
    return mT2, cvec2, wvb


_NC = None


def kernel(**inputs):
    out, _ = run_kernel(inputs)
    return out


def run_kernel(inputs, **spmd_kwargs):
    global _NC
    if _NC is None:
        _NC = build_nc()

    query = np.ascontiguousarray(np.asarray(inputs["query"], np.float32))
    key = np.ascontiguousarray(np.asarray(inputs["key"], np.float32))
    value = np.ascontiguousarray(np.asarray(inputs["value"], np.float32))
    mT2, cvec2, wvb = host_consts(
        np.asarray(inputs["wq"], np.float32), np.asarray(inputs["bq"], np.float32),
        np.asarray(inputs["wk"], np.float32), np.asarray(inputs["bk"], np.float32),
        np.asarray(inputs["wv"], np.float32), np.asarray(inputs["bv"], np.float32))

    B = query.shape[0]
    in_maps = [
        {
            "query": query[b], "key": key[b], "value": value[b],
            "mT2": mT2, "cvec2": cvec2, "wvb": wvb,
        }
        for b in range(B)
    ]
    res = run_bass_kernel_spmd(_NC, in_maps, core_ids=list(range(B)), **spmd_kwargs)
    out = np.stack([res.results[b]["out"] for b in range(B)]).astype(np.float32)
    return out, res
